# revision 40
# baseline (speedup 1.0000x reference)
"""TRN2 Bass kernel for nn_MultiHeadSeqAttention (B=8, M=1024, H=1024, 16 heads).

Reference computes out = ((h Wq^T) (h Wk^T)^T) (h Wv^T) per head, then Wo^T.
There is NO softmax, so the attention product reassociates:
    out_h = q_h @ (k_h^T @ v_h)        with  k_h^T v_h : [64, 64]
which removes the [M, M] score matrix entirely. Per core (1 batch):
4 dense 1024^3 GEMMs + tiny per-head [64x64] products, ~8.9 GFLOP.

Sharding: data-parallel over B across 8 cores; no collectives.
Host pre-transposes h and the weights so every matmul contraction is on the
partition dim (no on-chip transposes).

Schedule (v3):
 - k-projection runs ci-OUTER with 7 accumulation chains open across 7 PSUM
   banks, with ht tiles loaded in column halves, so the first real matmul
   fires once 256KB (ht0-half + wk0-half) lands instead of the full 4MB.
 - S is computed TRANSPOSED (v^T k) and folded into the output projection:
   woS_h = S_h^T-block-diag @ wo  (16 full-width matmuls), so phase 2 is a
   pure q-projection and phase 3 contracts qT directly with woS. No o-stage,
   no phase-boundary stall.
 - output is stored bf16 (host upcasts); halves the out DMA.

Precision: bf16 operands everywhere (fp8 E4M3 measured 3.3-4e-2 absmax rel
err for even one GEMM -- over the 2e-2 budget), fp32 accumulate.

Per-core layouts (i = input feature, o = q/k/v feature, j = out feature):
    ht  [i, m] = h[b].T          wq/wk/wv [i, o] = W.T        wo [o, j] = Wo.T
    k = ht.T @ wk : [m, o]       v : [m, o]      (partition = m)
    St_h = v_h^T @ k_h : [64,64] packed in two [128,512] PSUM banks
    woS = blkdiag(St_h) @ wo : [d, j]   qT = wq.T @ ht : [d, m]
    out = qT.T @ woS : [m, j]    (partition = m, DMA'd straight out)
"""

import numpy as np
import ml_dtypes

import concourse.bass as bass
import concourse.mybir as mybir
import concourse.tile as tile
from concourse import bacc
from concourse.bass_utils import run_bass_kernel_spmd

F32 = mybir.dt.float32
BF16 = mybir.dt.bfloat16

P = 128          # partitions
H = 1024         # model dim
M = 1024         # sequence length
NT = H // P      # 8 tiles of 128 along any 1024 dim
D = 64           # head dim
NH = 16          # heads
NC = 8           # cores
FD = 512         # matmul moving free dim
WARMUP_MM = 6    # PE warmup matmuls (bridge until the first DMA pair lands)

_CACHE = {}


def _build():
    nc = bacc.Bacc("TRN2", target_bir_lowering=False, debug=False,
                   num_devices=NC, enable_asserts=False,
                   enable_partition_id=False)

    ht_d = nc.dram_tensor("ht", [H, M], BF16, kind="ExternalInput")
    wq_d = nc.dram_tensor("wq", [H, H], BF16, kind="ExternalInput")
    wk_d = nc.dram_tensor("wk", [H, H], BF16, kind="ExternalInput")
    wv_d = nc.dram_tensor("wv", [H, H], BF16, kind="ExternalInput")
    wo_d = nc.dram_tensor("wo", [H, H], BF16, kind="ExternalInput")
    out_d = nc.dram_tensor("out", [M, H], BF16, kind="ExternalOutput")

    with tile.TileContext(nc) as tc:
        with tc.tile_pool(name="sb", bufs=1) as sb, \
             tc.tile_pool(name="ps", bufs=1, space="PSUM") as ps:

            # ---- PE warmup on bank 7: dep-free matmuls issued while the
            # first DMAs stream in. Operands are framework const tensors ----
            wu_lhs = nc.const_aps.tensor(1.0, [P, P], BF16)
            wu_rhs = nc.const_aps.tensor(1.0, [P, FD], BF16)
            wu_ps = ps.tile([P, FD], F32, tag="b7", name="wu_ps")
            for _ in range(WARMUP_MM):
                nc.tensor.matmul(wu_ps[:], wu_lhs, wu_rhs,
                                 start=True, stop=True)

            # ---- loads: ramp-critical chunks interleaved across BOTH HW
            # DMA queues in the need-order of the ci-outer chains (pair ci =
            # ht_ci + wk_ci half0); the scalar HWDGE queue spins up ~1us
            # later than sync, so it gets the later-needed half of each
            # pair. wk half1 (co=1 chains), wv, wq, wo stream in after ----
            ht_s = [sb.tile([P, M], BF16, tag=f"ht{i}", name=f"ht{i}")
                    for i in range(NT)]
            wk_s = [sb.tile([P, H], BF16, tag=f"wa{i}", name=f"wk{i}")
                    for i in range(NT)]

            def ld_ht(q, i, lo=0, hi=M):
                q.dma_start(ht_s[i][:, lo:hi],
                            ht_d.ap()[P * i:P * i + P, lo:hi])

            def ld_w(q, dst, src, i, lo=0, hi=H):
                q.dma_start(dst[i][:, lo:hi], src.ap()[P * i:P * i + P, lo:hi])

            # tiny head transfer on scalar: absorbs the queue's first-packet
            # spin-up latency so the real ramp chunks stream right behind
            ld_ht(nc.sync, 0, 0, FD)
            ld_ht(nc.scalar, 0, FD, FD + 16)
            ld_ht(nc.scalar, 0, FD + 16, M)
            ld_w(nc.sync, wk_s, wk_d, 0, 0, FD)
            ld_w(nc.scalar, wk_s, wk_d, 1, 0, FD)
            for i in range(1, NT):
                ld_ht(nc.sync if i % 2 else nc.scalar, i)
                if i + 1 < NT:
                    # opposite parity from ht_{i+1}: each (ht, wk-h0) pair
                    # is split across the two queues, so pairs complete in
                    # lockstep instead of alternating whole-pair per queue
                    ld_w(nc.sync if i % 2 else nc.scalar,
                         wk_s, wk_d, i + 1, 0, FD)
            for i in range(NT):
                nc.scalar.dma_start(wk_s[i][:, FD:H],
                                    wk_d.ap()[P * i:P * i + P, FD:H])
            wv_s, wq_s = [], []
            for i in range(NT):
                v_t = sb.tile([P, H], BF16, tag=f"wb{i}", name=f"wv{i}")
                nc.scalar.dma_start(v_t[:], wv_d.ap()[P * i:P * i + P, :])
                wv_s.append(v_t)
                q_t = sb.tile([P, H], BF16, tag=f"wq{i}", name=f"wq{i}")
                nc.sync.dma_start(q_t[:], wq_d.ap()[P * i:P * i + P, :])
                wq_s.append(q_t)

            # ---- phase 1a: k projection ----
            # ci-outer ramp: 7 chains (tm=0..6, co=0) open on banks 0-6; MM
            # (tm, ci) needs only the DMA chunks of pair ci; chains 0-3 read
            # the ht_ci first half, 4-6 the second.
            k_tiles = [sb.tile([P, H], BF16, tag=f"k{t}", name=f"k{t}")
                       for t in range(NT)]
            acc = [ps.tile([P, FD], F32, tag=f"b{t}", name=f"ka{t}")
                   for t in range(7)]
            for ci in range(NT):
                for t in range(7):
                    nc.tensor.matmul(
                        acc[t][:],
                        ht_s[ci][:, P * t:P * t + P],
                        wk_s[ci][:, 0:FD],
                        start=(ci == 0), stop=(ci == NT - 1),
                        skip_group_check=True,
                    )
                # dep-free filler between early groups: absorbs DMA-arrival
                # jitter and keeps the HAM activity window alive if a chunk
                # is late (an idle gap here delays the 2.4GHz unthrottle)
                if ci < 2:
                    for _ in range(2 - ci):
                        nc.tensor.matmul(wu_ps[:], wu_lhs, wu_rhs,
                                         start=True, stop=True)
            for t in range(7):
                nc.vector.tensor_copy(k_tiles[t][:, 0:FD], acc[t][:])

            # remaining k chains run sequentially (all data resident by now);
            # each bank's evacuation cast hides under the next chain
            def kchain(tm, co, bank):
                p_t = ps.tile([P, FD], F32, tag=bank, name=f"kb{tm}_{co}")
                for ci in range(NT):
                    nc.tensor.matmul(
                        p_t[:],
                        ht_s[ci][:, P * tm:P * tm + P],
                        wk_s[ci][:, FD * co:FD * co + FD],
                        start=(ci == 0), stop=(ci == NT - 1),
                    )
                nc.vector.tensor_copy(
                    k_tiles[tm][:, FD * co:FD * co + FD], p_t[:])

            kchain(7, 0, "b7")
            for tm in range(NT):
                kchain(tm, 1, f"b{tm}")

            # ---- phase 1b: v projection + S^T accumulation ----
            # One matmul per (m-tile, head pair g): v_pair^T @ k_pair gives a
            # 2x2 head block; diagonal blocks are S_2g^T / S_2g+1^T. Pairs
            # 0-3 accumulate in s_psA, 4-7 in s_psB. Interleaved per-pair
            # groups share the banks; zero once, accumulate with start=False.
            s_psA = ps.tile([P, FD], F32, tag="b0", name="s_psA")
            s_psB = ps.tile([P, FD], F32, tag="b1", name="s_psB")
            nc.vector.memset(s_psA[:], 0.0)
            nc.vector.memset(s_psB[:], 0.0)
            vb = 0
            for tm in range(NT):
                k_t = k_tiles[tm]
                v_t = sb.tile([P, H], BF16, tag="vv", bufs=4, name=f"v{tm}")
                for co in range(2):
                    p_t = ps.tile([P, FD], F32, tag=f"b{2 + vb % 6}",
                                  name=f"pv{tm}_{co}")
                    vb += 1
                    for ci in range(NT):
                        nc.tensor.matmul(
                            p_t[:],
                            ht_s[ci][:, P * tm:P * tm + P],
                            wv_s[ci][:, FD * co:FD * co + FD],
                            start=(ci == 0), stop=(ci == NT - 1),
                        )
                    nc.vector.tensor_copy(v_t[:, FD * co:FD * co + FD], p_t[:])
                    for g in range(4 * co, 4 * co + 4):
                        bank = s_psA if g < 4 else s_psB
                        cc = P * (g % 4)
                        nc.tensor.matmul(
                            bank[:, cc:cc + P],
                            v_t[:, P * g:P * g + P],
                            k_t[:, P * g:P * g + P],
                            start=False, stop=(tm == NT - 1),
                            skip_group_check=True,
                        )

            # ---- wo loads (reuse wa slots once wk is done) ----
            wo_s = []
            for i in range(NT):
                o_t = sb.tile([P, H], BF16, tag=f"wa{i}", name=f"wo{i}")
                (nc.sync if i % 2 else nc.scalar).dma_start(
                    o_t[:], wo_d.ap()[P * i:P * i + P, :])
                wo_s.append(o_t)

            # ---- block-diagonal S^T tiles: bd[g] = diag(S_2g^T, S_2g+1^T)
            # zero-padded, the lhsT of the wo-fold matmuls ----
            bd = []
            for g in range(NT):
                b_t = sb.tile([P, P], BF16, tag=f"bd{g}", name=f"bd{g}")
                nc.vector.memset(b_t[:], 0.0)
                s_bank = s_psA if g < 4 else s_psB
                cc = P * (g % 4)
                nc.vector.tensor_copy(b_t[0:D, 0:D], s_bank[0:D, cc:cc + D])
                nc.vector.tensor_copy(b_t[D:P, D:P],
                                      s_bank[D:P, cc + D:cc + P])
                bd.append(b_t)

            # ---- phase 2: qT projection; the wo-fold matmuls
            # (woS = blkdiag(S^T) @ wo) slot in after the first q chains so
            # the vector engine has time to build the bd tiles ----
            qt_tiles = [None] * NT
            pb = [0]

            def nextbank():
                t = f"b{2 + pb[0] % 6}"
                pb[0] += 1
                return t

            def emit_q(to):
                q_t = sb.tile([P, M], BF16, tag=f"qt{to}", name=f"qt{to}")
                for cm in range(2):
                    p_t = ps.tile([P, FD], F32, tag=nextbank(),
                                  name=f"pq{to}_{cm}")
                    for ci in range(NT):
                        nc.tensor.matmul(
                            p_t[:],
                            wq_s[ci][:, P * to:P * to + P],
                            ht_s[ci][:, FD * cm:FD * cm + FD],
                            start=(ci == 0), stop=(ci == NT - 1),
                        )
                    nc.vector.tensor_copy(q_t[:, FD * cm:FD * cm + FD], p_t[:])
                qt_tiles[to] = q_t

            ws_tiles = []

            def emit_ws(to):
                w_t = sb.tile([P, H], BF16, tag=f"ws{to}", name=f"ws{to}")
                for cj in range(2):
                    p_t = ps.tile([P, FD], F32, tag=nextbank(),
                                  name=f"pw{to}_{cj}")
                    nc.tensor.matmul(
                        p_t[:],
                        bd[to][:],
                        wo_s[to][:, FD * cj:FD * cj + FD],
                        start=True, stop=True,
                    )
                    nc.vector.tensor_copy(w_t[:, FD * cj:FD * cj + FD], p_t[:])
                ws_tiles.append(w_t)

            # interleave: each ws pair (0.4us of PE, 1.4us of casts) hides
            # under a q chain (3.5us of PE) so bank evacuations keep up
            emit_q(0)
            emit_q(1)
            emit_ws(0), emit_ws(1)
            emit_q(2)
            emit_ws(2), emit_ws(3)
            emit_q(3)
            emit_ws(4), emit_ws(5)
            emit_q(4)
            emit_ws(6), emit_ws(7)
            for to in range(5, NT):
                emit_q(to)

            # ---- phase 3: out = qT.T @ woS (bf16 out, host upcasts) ----
            # the very last 512-col chunk runs as two [128,256] chains so the
            # first half's cast+store hides under the second half's matmuls
            # and the tail gates on a [128,256] cast + 64KB transfer
            ob = [0]

            def out_chain(tm, lo, hi, q, cast_eng=None):
                o_sb = out_sb[tm]
                p_t = ps.tile([P, hi - lo], F32, tag=f"b{ob[0] % 8}",
                              name=f"pf{tm}_{lo}")
                ob[0] += 1
                for to in range(NT):
                    nc.tensor.matmul(
                        p_t[:],
                        qt_tiles[to][:, P * tm:P * tm + P],
                        ws_tiles[to][:, lo:hi],
                        start=(to == 0), stop=(to == NT - 1),
                    )
                if cast_eng is None:
                    nc.vector.tensor_copy(o_sb[:, lo:hi], p_t[:])
                else:
                    cast_eng.copy(o_sb[:, lo:hi], p_t[:])
                q.dma_start(out_d.ap()[P * tm:P * tm + P, lo:hi],
                            o_sb[:, lo:hi])

            out_sb = [sb.tile([P, H], BF16, tag=f"wb{t}", name=f"osb{t}")
                      for t in range(NT)]
            for tm in range(NT):
                out_chain(tm, 0, FD, nc.scalar)
                if tm == NT - 1:
                    out_chain(tm, FD, FD + 256, nc.sync)
                    out_chain(tm, FD + 256, FD + 384, nc.scalar,
                              cast_eng=nc.scalar)
                    out_chain(tm, FD + 384, H, nc.sync)
                else:
                    out_chain(tm, FD, H, nc.sync)

    nc.compile()
    return nc


def _get_nc():
    if "nc" not in _CACHE:
        _CACHE["nc"] = _build()
    return _CACHE["nc"]


def _run(h, Wq, Wk, Wv, Wo, trace=False):
    nc = _get_nc()
    bf16 = ml_dtypes.bfloat16
    wq = np.ascontiguousarray(np.asarray(Wq).T).astype(bf16)
    wk = np.ascontiguousarray(np.asarray(Wk).T).astype(bf16)
    wv = np.ascontiguousarray(np.asarray(Wv).T).astype(bf16)
    wo = np.ascontiguousarray(np.asarray(Wo).T).astype(bf16)
    in_maps = []
    for b in range(NC):
        in_maps.append({
            "ht": np.ascontiguousarray(np.asarray(h[b]).T).astype(bf16),
            "wq": wq, "wk": wk, "wv": wv, "wo": wo,
        })
    res = run_bass_kernel_spmd(nc, in_maps, core_ids=list(range(NC)), trace=trace)
    out = np.stack(
        [np.asarray(res.results[b]["out"]).astype(np.float32)
         for b in range(NC)], axis=0)
    return out, res


def kernel(h, key_pe, Wq, Wk, Wv, Wo):
    # key_pe only feeds the reference's dead softmax branch; unused.
    out, _ = _run(h, Wq, Wk, Wv, Wo)
    return out


# revision 43
# speedup vs baseline: 1.0063x; 1.0063x over previous
"""TRN2 Bass kernel for nn_MultiHeadSeqAttention (B=8, M=1024, H=1024, 16 heads).

Reference computes out = ((h Wq^T) (h Wk^T)^T) (h Wv^T) per head, then Wo^T.
There is NO softmax, so the attention product reassociates:
    out_h = q_h @ (k_h^T @ v_h)        with  k_h^T v_h : [64, 64]
which removes the [M, M] score matrix entirely. Per core (1 batch):
4 dense 1024^3 GEMMs + tiny per-head [64x64] products, ~8.9 GFLOP.

Sharding: data-parallel over B across 8 cores; no collectives.
Host pre-transposes h and the weights so every matmul contraction is on the
partition dim (no on-chip transposes).

Schedule (v3):
 - k-projection runs ci-OUTER with 7 accumulation chains open across 7 PSUM
   banks, with ht tiles loaded in column halves, so the first real matmul
   fires once 256KB (ht0-half + wk0-half) lands instead of the full 4MB.
 - S is computed TRANSPOSED (v^T k) and folded into the output projection:
   woS_h = S_h^T-block-diag @ wo  (16 full-width matmuls), so phase 2 is a
   pure q-projection and phase 3 contracts qT directly with woS. No o-stage,
   no phase-boundary stall.
 - output is stored bf16 (host upcasts); halves the out DMA.

Precision: bf16 operands everywhere (fp8 E4M3 measured 3.3-4e-2 absmax rel
err for even one GEMM -- over the 2e-2 budget), fp32 accumulate.

Per-core layouts (i = input feature, o = q/k/v feature, j = out feature):
    ht  [i, m] = h[b].T          wq/wk/wv [i, o] = W.T        wo [o, j] = Wo.T
    k = ht.T @ wk : [m, o]       v : [m, o]      (partition = m)
    St_h = v_h^T @ k_h : [64,64] packed in two [128,512] PSUM banks
    woS = blkdiag(St_h) @ wo : [d, j]   qT = wq.T @ ht : [d, m]
    out = qT.T @ woS : [m, j]    (partition = m, DMA'd straight out)
"""

import numpy as np
import ml_dtypes

import concourse.bass as bass
import concourse.mybir as mybir
import concourse.tile as tile
from concourse import bacc
from concourse.bass_utils import run_bass_kernel_spmd

F32 = mybir.dt.float32
BF16 = mybir.dt.bfloat16

P = 128          # partitions
H = 1024         # model dim
M = 1024         # sequence length
NT = H // P      # 8 tiles of 128 along any 1024 dim
D = 64           # head dim
NH = 16          # heads
NC = 8           # cores
FD = 512         # matmul moving free dim
WARMUP_MM = 6    # PE warmup matmuls (bridge until the first DMA pair lands)

_CACHE = {}


def _build():
    nc = bacc.Bacc("TRN2", target_bir_lowering=False, debug=False,
                   num_devices=NC, enable_asserts=False,
                   enable_partition_id=False)

    ht_d = nc.dram_tensor("ht", [H, M], BF16, kind="ExternalInput")
    wq_d = nc.dram_tensor("wq", [H, H], BF16, kind="ExternalInput")
    wk_d = nc.dram_tensor("wk", [H, H], BF16, kind="ExternalInput")
    wv_d = nc.dram_tensor("wv", [H, H], BF16, kind="ExternalInput")
    wo_d = nc.dram_tensor("wo", [H, H], BF16, kind="ExternalInput")
    out_d = nc.dram_tensor("out", [M, H], BF16, kind="ExternalOutput")

    with tile.TileContext(nc) as tc:
        with tc.tile_pool(name="sb", bufs=1) as sb, \
             tc.tile_pool(name="ps", bufs=1, space="PSUM") as ps:

            # ---- PE warmup on bank 7: dep-free matmuls issued while the
            # first DMAs stream in. Operands are framework const tensors ----
            wu_lhs = nc.const_aps.tensor(1.0, [P, P], BF16)
            wu_rhs = nc.const_aps.tensor(1.0, [P, FD], BF16)
            wu_ps = ps.tile([P, FD], F32, tag="b7", name="wu_ps")
            for _ in range(WARMUP_MM):
                nc.tensor.matmul(wu_ps[:], wu_lhs, wu_rhs,
                                 start=True, stop=True)

            # ---- loads: ramp-critical chunks interleaved across BOTH HW
            # DMA queues in the need-order of the ci-outer chains (pair ci =
            # ht_ci + wk_ci half0); the scalar HWDGE queue spins up ~1us
            # later than sync, so it gets the later-needed half of each
            # pair. wk half1 (co=1 chains), wv, wq, wo stream in after ----
            ht_s = [sb.tile([P, M], BF16, tag=f"ht{i}", name=f"ht{i}")
                    for i in range(NT)]
            wk_s = [sb.tile([P, H], BF16, tag=f"wa{i}", name=f"wk{i}")
                    for i in range(NT)]

            def ld_ht(q, i, lo=0, hi=M):
                q.dma_start(ht_s[i][:, lo:hi],
                            ht_d.ap()[P * i:P * i + P, lo:hi])

            def ld_w(q, dst, src, i, lo=0, hi=H):
                q.dma_start(dst[i][:, lo:hi], src.ap()[P * i:P * i + P, lo:hi])

            # tiny head transfer on scalar: absorbs the queue's first-packet
            # spin-up latency so the real ramp chunks stream right behind
            ld_ht(nc.sync, 0, 0, FD)
            ld_ht(nc.scalar, 0, FD, FD + 16)
            ld_ht(nc.scalar, 0, FD + 16, M)
            ld_w(nc.sync, wk_s, wk_d, 0, 0, FD)
            ld_w(nc.scalar, wk_s, wk_d, 1, 0, FD)
            for i in range(1, NT):
                ld_ht(nc.sync if i % 2 else nc.scalar, i)
                if i + 1 < NT:
                    # opposite parity from ht_{i+1}: each (ht, wk-h0) pair
                    # is split across the two queues, so pairs complete in
                    # lockstep instead of alternating whole-pair per queue
                    ld_w(nc.sync if i % 2 else nc.scalar,
                         wk_s, wk_d, i + 1, 0, FD)
            for i in range(NT):
                nc.scalar.dma_start(wk_s[i][:, FD:H],
                                    wk_d.ap()[P * i:P * i + P, FD:H])
            wv_s, wq_s = [], []
            for i in range(NT):
                v_t = sb.tile([P, H], BF16, tag=f"wb{i}", name=f"wv{i}")
                nc.scalar.dma_start(v_t[:], wv_d.ap()[P * i:P * i + P, :])
                wv_s.append(v_t)
                q_t = sb.tile([P, H], BF16, tag=f"wq{i}", name=f"wq{i}")
                nc.sync.dma_start(q_t[:], wq_d.ap()[P * i:P * i + P, :])
                wq_s.append(q_t)

            # ---- phase 1a: k projection ----
            # ci-outer ramp: 7 chains (tm=0..6, co=0) open on banks 0-6; MM
            # (tm, ci) needs only the DMA chunks of pair ci; chains 0-3 read
            # the ht_ci first half, 4-6 the second.
            k_tiles = [sb.tile([P, H], BF16, tag=f"k{t}", name=f"k{t}")
                       for t in range(NT)]
            acc = [ps.tile([P, FD], F32, tag=f"b{t}", name=f"ka{t}")
                   for t in range(7)]
            for ci in range(NT):
                for t in range(7):
                    nc.tensor.matmul(
                        acc[t][:],
                        ht_s[ci][:, P * t:P * t + P],
                        wk_s[ci][:, 0:FD],
                        start=(ci == 0), stop=(ci == NT - 1),
                        skip_group_check=True,
                    )
                # dep-free filler between early groups: absorbs DMA-arrival
                # jitter and keeps the HAM activity window alive if a chunk
                # is late (an idle gap here delays the 2.4GHz unthrottle)
                if ci < 2:
                    for _ in range(2 - ci):
                        nc.tensor.matmul(wu_ps[:], wu_lhs, wu_rhs,
                                         start=True, stop=True)
            for t in range(7):
                nc.vector.tensor_copy(k_tiles[t][:, 0:FD], acc[t][:])

            # remaining k chains run sequentially (all data resident by now);
            # each bank's evacuation cast hides under the next chain
            def kchain(tm, co, bank):
                p_t = ps.tile([P, FD], F32, tag=bank, name=f"kb{tm}_{co}")
                for ci in range(NT):
                    nc.tensor.matmul(
                        p_t[:],
                        ht_s[ci][:, P * tm:P * tm + P],
                        wk_s[ci][:, FD * co:FD * co + FD],
                        start=(ci == 0), stop=(ci == NT - 1),
                    )
                nc.vector.tensor_copy(
                    k_tiles[tm][:, FD * co:FD * co + FD], p_t[:])

            kchain(7, 0, "b7")
            for tm in range(NT):
                kchain(tm, 1, f"b{tm}")

            # ---- phase 1b: v projection + S^T accumulation ----
            # One matmul per (m-tile, head pair g): v_pair^T @ k_pair gives a
            # 2x2 head block; diagonal blocks are S_2g^T / S_2g+1^T. Pairs
            # 0-3 accumulate in s_psA, 4-7 in s_psB. Interleaved per-pair
            # groups share the banks; zero once, accumulate with start=False.
            s_psA = ps.tile([P, FD], F32, tag="b0", name="s_psA")
            s_psB = ps.tile([P, FD], F32, tag="b1", name="s_psB")
            nc.vector.memset(s_psA[:], 0.0)
            nc.vector.memset(s_psB[:], 0.0)
            vb = 0
            for tm in range(NT):
                k_t = k_tiles[tm]
                v_t = sb.tile([P, H], BF16, tag="vv", bufs=4, name=f"v{tm}")
                for co in range(2):
                    p_t = ps.tile([P, FD], F32, tag=f"b{2 + vb % 6}",
                                  name=f"pv{tm}_{co}")
                    vb += 1
                    for ci in range(NT):
                        nc.tensor.matmul(
                            p_t[:],
                            ht_s[ci][:, P * tm:P * tm + P],
                            wv_s[ci][:, FD * co:FD * co + FD],
                            start=(ci == 0), stop=(ci == NT - 1),
                        )
                    nc.vector.tensor_copy(v_t[:, FD * co:FD * co + FD], p_t[:])
                    for g in range(4 * co, 4 * co + 4):
                        bank = s_psA if g < 4 else s_psB
                        cc = P * (g % 4)
                        nc.tensor.matmul(
                            bank[:, cc:cc + P],
                            v_t[:, P * g:P * g + P],
                            k_t[:, P * g:P * g + P],
                            start=False, stop=(tm == NT - 1),
                            skip_group_check=True,
                        )

            # ---- wo loads (reuse wa slots once wk is done) ----
            wo_s = []
            for i in range(NT):
                o_t = sb.tile([P, H], BF16, tag=f"wa{i}", name=f"wo{i}")
                (nc.sync if i % 2 else nc.scalar).dma_start(
                    o_t[:], wo_d.ap()[P * i:P * i + P, :])
                wo_s.append(o_t)

            # ---- block-diagonal S^T tiles: bd[g] = diag(S_2g^T, S_2g+1^T)
            # zero-padded, the lhsT of the wo-fold matmuls ----
            bd = []
            for g in range(NT):
                b_t = sb.tile([P, P], BF16, tag=f"bd{g}", name=f"bd{g}")
                nc.vector.memset(b_t[:], 0.0)
                s_bank = s_psA if g < 4 else s_psB
                cc = P * (g % 4)
                nc.vector.tensor_copy(b_t[0:D, 0:D], s_bank[0:D, cc:cc + D])
                nc.vector.tensor_copy(b_t[D:P, D:P],
                                      s_bank[D:P, cc + D:cc + P])
                bd.append(b_t)

            # ---- phase 2: qT projection; the wo-fold matmuls
            # (woS = blkdiag(S^T) @ wo) slot in after the first q chains so
            # the vector engine has time to build the bd tiles ----
            qt_tiles = [None] * NT
            pb = [0]

            def nextbank():
                t = f"b{2 + pb[0] % 6}"
                pb[0] += 1
                return t

            def emit_q(to):
                q_t = sb.tile([P, M], BF16, tag=f"qt{to}", name=f"qt{to}")
                for cm in range(2):
                    p_t = ps.tile([P, FD], F32, tag=nextbank(),
                                  name=f"pq{to}_{cm}")
                    for ci in range(NT):
                        nc.tensor.matmul(
                            p_t[:],
                            wq_s[ci][:, P * to:P * to + P],
                            ht_s[ci][:, FD * cm:FD * cm + FD],
                            start=(ci == 0), stop=(ci == NT - 1),
                        )
                    nc.vector.tensor_copy(q_t[:, FD * cm:FD * cm + FD], p_t[:])
                qt_tiles[to] = q_t

            ws_tiles = []

            def emit_ws(to):
                w_t = sb.tile([P, H], BF16, tag=f"ws{to}", name=f"ws{to}")
                for cj in range(2):
                    p_t = ps.tile([P, FD], F32, tag=nextbank(),
                                  name=f"pw{to}_{cj}")
                    nc.tensor.matmul(
                        p_t[:],
                        bd[to][:],
                        wo_s[to][:, FD * cj:FD * cj + FD],
                        start=True, stop=True,
                    )
                    nc.vector.tensor_copy(w_t[:, FD * cj:FD * cj + FD], p_t[:])
                ws_tiles.append(w_t)

            # interleave: each ws pair (0.4us of PE, 1.4us of casts) hides
            # under a q chain (3.5us of PE) so bank evacuations keep up
            emit_q(0)
            emit_q(1)
            emit_ws(0), emit_ws(1)
            emit_q(2)
            emit_ws(2), emit_ws(3)
            emit_q(3)
            emit_ws(4), emit_ws(5)
            emit_q(4)
            emit_ws(6), emit_ws(7)
            for to in range(5, NT):
                emit_q(to)

            # ---- phase 3: out = qT.T @ woS (bf16 out, host upcasts) ----
            # the very last 512-col chunk runs as two [128,256] chains so the
            # first half's cast+store hides under the second half's matmuls
            # and the tail gates on a [128,256] cast + 64KB transfer
            ob = [0]

            def out_chain(tm, lo, hi, q, cast_eng=None):
                o_sb = out_sb[tm]
                p_t = ps.tile([P, hi - lo], F32, tag=f"b{ob[0] % 8}",
                              name=f"pf{tm}_{lo}")
                ob[0] += 1
                for to in range(NT):
                    nc.tensor.matmul(
                        p_t[:],
                        qt_tiles[to][:, P * tm:P * tm + P],
                        ws_tiles[to][:, lo:hi],
                        start=(to == 0), stop=(to == NT - 1),
                    )
                if cast_eng is None:
                    nc.vector.tensor_copy(o_sb[:, lo:hi], p_t[:])
                else:
                    cast_eng.copy(o_sb[:, lo:hi], p_t[:])
                q.dma_start(out_d.ap()[P * tm:P * tm + P, lo:hi],
                            o_sb[:, lo:hi])

            out_sb = [sb.tile([P, H], BF16, tag=f"wb{t}", name=f"osb{t}")
                      for t in range(NT)]
            for tm in range(NT):
                out_chain(tm, 0, FD, nc.scalar)
                if tm == NT - 1:
                    # all tail casts on vector: a single scalar-engine copy
                    # would pull a 1.3us ACT_TABLE_LOAD onto the scalar
                    # queue right at ramp start, delaying the ramp DMAs
                    out_chain(tm, FD, FD + 256, nc.sync)
                    out_chain(tm, FD + 256, FD + 384, nc.scalar)
                    out_chain(tm, FD + 384, H, nc.sync)
                else:
                    out_chain(tm, FD, H, nc.sync)

    nc.compile()
    return nc


def _get_nc():
    if "nc" not in _CACHE:
        _CACHE["nc"] = _build()
    return _CACHE["nc"]


def _run(h, Wq, Wk, Wv, Wo, trace=False):
    nc = _get_nc()
    bf16 = ml_dtypes.bfloat16
    wq = np.ascontiguousarray(np.asarray(Wq).T).astype(bf16)
    wk = np.ascontiguousarray(np.asarray(Wk).T).astype(bf16)
    wv = np.ascontiguousarray(np.asarray(Wv).T).astype(bf16)
    wo = np.ascontiguousarray(np.asarray(Wo).T).astype(bf16)
    in_maps = []
    for b in range(NC):
        in_maps.append({
            "ht": np.ascontiguousarray(np.asarray(h[b]).T).astype(bf16),
            "wq": wq, "wk": wk, "wv": wv, "wo": wo,
        })
    res = run_bass_kernel_spmd(nc, in_maps, core_ids=list(range(NC)), trace=trace)
    out = np.stack(
        [np.asarray(res.results[b]["out"]).astype(np.float32)
         for b in range(NC)], axis=0)
    return out, res


def kernel(h, key_pe, Wq, Wk, Wv, Wo):
    # key_pe only feeds the reference's dead softmax branch; unused.
    out, _ = _run(h, Wq, Wk, Wv, Wo)
    return out


# revision 44
# speedup vs baseline: 1.0082x; 1.0019x over previous
"""TRN2 Bass kernel for nn_MultiHeadSeqAttention (B=8, M=1024, H=1024, 16 heads).

Reference computes out = ((h Wq^T) (h Wk^T)^T) (h Wv^T) per head, then Wo^T.
There is NO softmax, so the attention product reassociates:
    out_h = q_h @ (k_h^T @ v_h)        with  k_h^T v_h : [64, 64]
which removes the [M, M] score matrix entirely. Per core (1 batch):
4 dense 1024^3 GEMMs + tiny per-head [64x64] products, ~8.9 GFLOP.

Sharding: data-parallel over B across 8 cores; no collectives.
Host pre-transposes h and the weights so every matmul contraction is on the
partition dim (no on-chip transposes).

Schedule (v3):
 - k-projection runs ci-OUTER with 7 accumulation chains open across 7 PSUM
   banks, with ht tiles loaded in column halves, so the first real matmul
   fires once 256KB (ht0-half + wk0-half) lands instead of the full 4MB.
 - S is computed TRANSPOSED (v^T k) and folded into the output projection:
   woS_h = S_h^T-block-diag @ wo  (16 full-width matmuls), so phase 2 is a
   pure q-projection and phase 3 contracts qT directly with woS. No o-stage,
   no phase-boundary stall.
 - output is stored bf16 (host upcasts); halves the out DMA.

Precision: bf16 operands everywhere (fp8 E4M3 measured 3.3-4e-2 absmax rel
err for even one GEMM -- over the 2e-2 budget), fp32 accumulate.

Per-core layouts (i = input feature, o = q/k/v feature, j = out feature):
    ht  [i, m] = h[b].T          wq/wk/wv [i, o] = W.T        wo [o, j] = Wo.T
    k = ht.T @ wk : [m, o]       v : [m, o]      (partition = m)
    St_h = v_h^T @ k_h : [64,64] packed in two [128,512] PSUM banks
    woS = blkdiag(St_h) @ wo : [d, j]   qT = wq.T @ ht : [d, m]
    out = qT.T @ woS : [m, j]    (partition = m, DMA'd straight out)
"""

import numpy as np
import ml_dtypes

import concourse.bass as bass
import concourse.mybir as mybir
import concourse.tile as tile
from concourse import bacc
from concourse.bass_utils import run_bass_kernel_spmd

F32 = mybir.dt.float32
BF16 = mybir.dt.bfloat16

P = 128          # partitions
H = 1024         # model dim
M = 1024         # sequence length
NT = H // P      # 8 tiles of 128 along any 1024 dim
D = 64           # head dim
NH = 16          # heads
NC = 8           # cores
FD = 512         # matmul moving free dim
WARMUP_MM = 6    # PE warmup matmuls (bridge until the first DMA pair lands)

_CACHE = {}


def _build():
    nc = bacc.Bacc("TRN2", target_bir_lowering=False, debug=False,
                   num_devices=NC, enable_asserts=False,
                   enable_partition_id=False)

    ht_d = nc.dram_tensor("ht", [H, M], BF16, kind="ExternalInput")
    wq_d = nc.dram_tensor("wq", [H, H], BF16, kind="ExternalInput")
    wk_d = nc.dram_tensor("wk", [H, H], BF16, kind="ExternalInput")
    wv_d = nc.dram_tensor("wv", [H, H], BF16, kind="ExternalInput")
    wo_d = nc.dram_tensor("wo", [H, H], BF16, kind="ExternalInput")
    out_d = nc.dram_tensor("out", [M, H], BF16, kind="ExternalOutput")

    with tile.TileContext(nc) as tc:
        with tc.tile_pool(name="sb", bufs=1) as sb, \
             tc.tile_pool(name="ps", bufs=1, space="PSUM") as ps:

            # ---- PE warmup on bank 7: dep-free matmuls issued while the
            # first DMAs stream in. Operands are framework const tensors ----
            wu_lhs = nc.const_aps.tensor(1.0, [P, P], BF16)
            wu_rhs = nc.const_aps.tensor(1.0, [P, FD], BF16)
            wu_ps = ps.tile([P, FD], F32, tag="b7", name="wu_ps")
            for _ in range(WARMUP_MM):
                nc.tensor.matmul(wu_ps[:], wu_lhs, wu_rhs,
                                 start=True, stop=True)

            # ---- loads: ramp-critical chunks interleaved across BOTH HW
            # DMA queues in the need-order of the ci-outer chains (pair ci =
            # ht_ci + wk_ci half0); the scalar HWDGE queue spins up ~1us
            # later than sync, so it gets the later-needed half of each
            # pair. wk half1 (co=1 chains), wv, wq, wo stream in after ----
            ht_s = [sb.tile([P, M], BF16, tag=f"ht{i}", name=f"ht{i}")
                    for i in range(NT)]
            wk_s = [sb.tile([P, H], BF16, tag=f"wa{i}", name=f"wk{i}")
                    for i in range(NT)]

            def ld_ht(q, i, lo=0, hi=M):
                q.dma_start(ht_s[i][:, lo:hi],
                            ht_d.ap()[P * i:P * i + P, lo:hi])

            def ld_w(q, dst, src, i, lo=0, hi=H):
                q.dma_start(dst[i][:, lo:hi], src.ap()[P * i:P * i + P, lo:hi])

            # tiny head transfer on scalar: absorbs the queue's first-packet
            # spin-up latency so the real ramp chunks stream right behind
            ld_ht(nc.sync, 0, 0, FD)
            ld_ht(nc.scalar, 0, FD, FD + 16)
            ld_ht(nc.scalar, 0, FD + 16, M)
            ld_w(nc.sync, wk_s, wk_d, 0, 0, FD)
            ld_w(nc.scalar, wk_s, wk_d, 1, 0, FD)
            for i in range(1, NT):
                ld_ht(nc.sync if i % 2 else nc.scalar, i)
                if i + 1 < NT:
                    # opposite parity from ht_{i+1}: each (ht, wk-h0) pair
                    # is split across the two queues, so pairs complete in
                    # lockstep instead of alternating whole-pair per queue
                    ld_w(nc.sync if i % 2 else nc.scalar,
                         wk_s, wk_d, i + 1, 0, FD)
            for i in range(NT):
                nc.scalar.dma_start(wk_s[i][:, FD:H],
                                    wk_d.ap()[P * i:P * i + P, FD:H])
            wv_s, wq_s = [], []
            for i in range(NT):
                v_t = sb.tile([P, H], BF16, tag=f"wb{i}", name=f"wv{i}")
                nc.scalar.dma_start(v_t[:], wv_d.ap()[P * i:P * i + P, :])
                wv_s.append(v_t)
                q_t = sb.tile([P, H], BF16, tag=f"wq{i}", name=f"wq{i}")
                nc.sync.dma_start(q_t[:], wq_d.ap()[P * i:P * i + P, :])
                wq_s.append(q_t)

            # ---- phase 1a: k projection ----
            # ci-outer ramp: 7 chains (tm=0..6, co=0) open on banks 0-6; MM
            # (tm, ci) needs only the DMA chunks of pair ci; chains 0-3 read
            # the ht_ci first half, 4-6 the second.
            k_tiles = [sb.tile([P, H], BF16, tag=f"k{t}", name=f"k{t}")
                       for t in range(NT)]
            acc = [ps.tile([P, FD], F32, tag=f"b{t}", name=f"ka{t}")
                   for t in range(7)]
            for ci in range(NT):
                for t in range(7):
                    nc.tensor.matmul(
                        acc[t][:],
                        ht_s[ci][:, P * t:P * t + P],
                        wk_s[ci][:, 0:FD],
                        start=(ci == 0), stop=(ci == NT - 1),
                        skip_group_check=True,
                    )
                # dep-free filler between early groups: absorbs DMA-arrival
                # jitter and keeps the HAM activity window alive if a chunk
                # is late (an idle gap here delays the 2.4GHz unthrottle)
                if ci < 2:
                    for _ in range(2 - ci):
                        nc.tensor.matmul(wu_ps[:], wu_lhs, wu_rhs,
                                         start=True, stop=True)
            for t in range(7):
                nc.vector.tensor_copy(k_tiles[t][:, 0:FD], acc[t][:])

            # remaining k chains run sequentially (all data resident by now);
            # each bank's evacuation cast hides under the next chain
            def kchain(tm, co, bank):
                p_t = ps.tile([P, FD], F32, tag=bank, name=f"kb{tm}_{co}")
                for ci in range(NT):
                    nc.tensor.matmul(
                        p_t[:],
                        ht_s[ci][:, P * tm:P * tm + P],
                        wk_s[ci][:, FD * co:FD * co + FD],
                        start=(ci == 0), stop=(ci == NT - 1),
                    )
                nc.vector.tensor_copy(
                    k_tiles[tm][:, FD * co:FD * co + FD], p_t[:])

            kchain(7, 0, "b7")
            for tm in range(NT):
                kchain(tm, 1, f"b{tm}")

            # ---- phase 1b: v projection + S^T accumulation ----
            # One matmul per (m-tile, head pair g): v_pair^T @ k_pair gives a
            # 2x2 head block; diagonal blocks are S_2g^T / S_2g+1^T. Pairs
            # 0-3 accumulate in s_psA, 4-7 in s_psB. Interleaved per-pair
            # groups share the banks; zero once, accumulate with start=False.
            s_psA = ps.tile([P, FD], F32, tag="b0", name="s_psA")
            s_psB = ps.tile([P, FD], F32, tag="b1", name="s_psB")
            nc.vector.memset(s_psA[:], 0.0)
            nc.vector.memset(s_psB[:], 0.0)
            vb = 0
            for tm in range(NT):
                k_t = k_tiles[tm]
                v_t = sb.tile([P, H], BF16, tag="vv", bufs=4, name=f"v{tm}")
                for co in range(2):
                    p_t = ps.tile([P, FD], F32, tag=f"b{2 + vb % 6}",
                                  name=f"pv{tm}_{co}")
                    vb += 1
                    for ci in range(NT):
                        nc.tensor.matmul(
                            p_t[:],
                            ht_s[ci][:, P * tm:P * tm + P],
                            wv_s[ci][:, FD * co:FD * co + FD],
                            start=(ci == 0), stop=(ci == NT - 1),
                        )
                    nc.vector.tensor_copy(v_t[:, FD * co:FD * co + FD], p_t[:])
                    for g in range(4 * co, 4 * co + 4):
                        bank = s_psA if g < 4 else s_psB
                        cc = P * (g % 4)
                        nc.tensor.matmul(
                            bank[:, cc:cc + P],
                            v_t[:, P * g:P * g + P],
                            k_t[:, P * g:P * g + P],
                            start=False, stop=(tm == NT - 1),
                            skip_group_check=True,
                        )

            # ---- wo loads (reuse wa slots once wk is done) ----
            wo_s = []
            for i in range(NT):
                o_t = sb.tile([P, H], BF16, tag=f"wa{i}", name=f"wo{i}")
                (nc.sync if i % 2 else nc.scalar).dma_start(
                    o_t[:], wo_d.ap()[P * i:P * i + P, :])
                wo_s.append(o_t)

            # ---- block-diagonal S^T tiles: bd[g] = diag(S_2g^T, S_2g+1^T)
            # zero-padded, the lhsT of the wo-fold matmuls ----
            bd = []
            for g in range(NT):
                b_t = sb.tile([P, P], BF16, tag=f"bd{g}", name=f"bd{g}")
                nc.vector.memset(b_t[:], 0.0)
                s_bank = s_psA if g < 4 else s_psB
                cc = P * (g % 4)
                nc.vector.tensor_copy(b_t[0:D, 0:D], s_bank[0:D, cc:cc + D])
                nc.vector.tensor_copy(b_t[D:P, D:P],
                                      s_bank[D:P, cc + D:cc + P])
                bd.append(b_t)

            # ---- phase 2: qT projection; the wo-fold matmuls
            # (woS = blkdiag(S^T) @ wo) slot in after the first q chains so
            # the vector engine has time to build the bd tiles ----
            qt_tiles = [None] * NT
            pb = [0]

            def nextbank():
                t = f"b{2 + pb[0] % 6}"
                pb[0] += 1
                return t

            def emit_q(to):
                q_t = sb.tile([P, M], BF16, tag=f"qt{to}", name=f"qt{to}")
                for cm in range(2):
                    p_t = ps.tile([P, FD], F32, tag=nextbank(),
                                  name=f"pq{to}_{cm}")
                    for ci in range(NT):
                        nc.tensor.matmul(
                            p_t[:],
                            wq_s[ci][:, P * to:P * to + P],
                            ht_s[ci][:, FD * cm:FD * cm + FD],
                            start=(ci == 0), stop=(ci == NT - 1),
                        )
                    nc.vector.tensor_copy(q_t[:, FD * cm:FD * cm + FD], p_t[:])
                qt_tiles[to] = q_t

            ws_tiles = []

            def emit_ws(to):
                w_t = sb.tile([P, H], BF16, tag=f"ws{to}", name=f"ws{to}")
                for cj in range(2):
                    p_t = ps.tile([P, FD], F32, tag=nextbank(),
                                  name=f"pw{to}_{cj}")
                    nc.tensor.matmul(
                        p_t[:],
                        bd[to][:],
                        wo_s[to][:, FD * cj:FD * cj + FD],
                        start=True, stop=True,
                    )
                    nc.vector.tensor_copy(w_t[:, FD * cj:FD * cj + FD], p_t[:])
                ws_tiles.append(w_t)

            # interleave: each ws pair (0.4us of PE, 1.4us of casts) hides
            # under a q chain (3.5us of PE) so bank evacuations keep up
            emit_q(0)
            emit_q(1)
            emit_ws(0), emit_ws(1)
            emit_q(2)
            emit_ws(2), emit_ws(3)
            emit_q(3)
            emit_ws(4), emit_ws(5)
            emit_q(4)
            emit_ws(6), emit_ws(7)
            for to in range(5, NT):
                emit_q(to)

            # ---- phase 3: out = qT.T @ woS (bf16 out, host upcasts) ----
            # the very last 512-col chunk runs as two [128,256] chains so the
            # first half's cast+store hides under the second half's matmuls
            # and the tail gates on a [128,256] cast + 64KB transfer
            ob = [0]

            def out_chain(tm, lo, hi, q, cast_eng=None):
                o_sb = out_sb[tm]
                p_t = ps.tile([P, hi - lo], F32, tag=f"b{ob[0] % 8}",
                              name=f"pf{tm}_{lo}")
                ob[0] += 1
                for to in range(NT):
                    nc.tensor.matmul(
                        p_t[:],
                        qt_tiles[to][:, P * tm:P * tm + P],
                        ws_tiles[to][:, lo:hi],
                        start=(to == 0), stop=(to == NT - 1),
                    )
                if cast_eng is None:
                    nc.vector.tensor_copy(o_sb[:, lo:hi], p_t[:])
                else:
                    cast_eng.copy(o_sb[:, lo:hi], p_t[:])
                q.dma_start(out_d.ap()[P * tm:P * tm + P, lo:hi],
                            o_sb[:, lo:hi])

            out_sb = [sb.tile([P, H], BF16, tag=f"wb{t}", name=f"osb{t}")
                      for t in range(NT)]
            for tm in range(NT):
                out_chain(tm, 0, FD, nc.scalar)
                if tm == NT - 1:
                    out_chain(tm, FD, FD + 256, nc.sync)
                    out_chain(tm, FD + 256, FD + 384, nc.scalar,
                              cast_eng=nc.scalar)
                    out_chain(tm, FD + 384, H, nc.sync)
                else:
                    out_chain(tm, FD, H, nc.sync)

    nc.compile()
    return nc


def _get_nc():
    if "nc" not in _CACHE:
        _CACHE["nc"] = _build()
    return _CACHE["nc"]


def _run(h, Wq, Wk, Wv, Wo, trace=False):
    nc = _get_nc()
    bf16 = ml_dtypes.bfloat16
    wq = np.ascontiguousarray(np.asarray(Wq).T).astype(bf16)
    wk = np.ascontiguousarray(np.asarray(Wk).T).astype(bf16)
    wv = np.ascontiguousarray(np.asarray(Wv).T).astype(bf16)
    wo = np.ascontiguousarray(np.asarray(Wo).T).astype(bf16)
    in_maps = []
    for b in range(NC):
        in_maps.append({
            "ht": np.ascontiguousarray(np.asarray(h[b]).T).astype(bf16),
            "wq": wq, "wk": wk, "wv": wv, "wo": wo,
        })
    res = run_bass_kernel_spmd(nc, in_maps, core_ids=list(range(NC)), trace=trace)
    out = np.stack(
        [np.asarray(res.results[b]["out"]).astype(np.float32)
         for b in range(NC)], axis=0)
    return out, res


def kernel(h, key_pe, Wq, Wk, Wv, Wo):
    # key_pe only feeds the reference's dead softmax branch; unused.
    out, _ = _run(h, Wq, Wk, Wv, Wo)
    return out


# revision 45
# speedup vs baseline: 1.0151x; 1.0068x over previous
"""TRN2 Bass kernel for nn_MultiHeadSeqAttention (B=8, M=1024, H=1024, 16 heads).

Reference computes out = ((h Wq^T) (h Wk^T)^T) (h Wv^T) per head, then Wo^T.
There is NO softmax, so the attention product reassociates:
    out_h = q_h @ (k_h^T @ v_h)        with  k_h^T v_h : [64, 64]
which removes the [M, M] score matrix entirely. Per core (1 batch):
4 dense 1024^3 GEMMs + tiny per-head [64x64] products, ~8.9 GFLOP.

Sharding: data-parallel over B across 8 cores; no collectives.
Host pre-transposes h and the weights so every matmul contraction is on the
partition dim (no on-chip transposes).

Schedule (v3):
 - k-projection runs ci-OUTER with 7 accumulation chains open across 7 PSUM
   banks, with ht tiles loaded in column halves, so the first real matmul
   fires once 256KB (ht0-half + wk0-half) lands instead of the full 4MB.
 - S is computed TRANSPOSED (v^T k) and folded into the output projection:
   woS_h = S_h^T-block-diag @ wo  (16 full-width matmuls), so phase 2 is a
   pure q-projection and phase 3 contracts qT directly with woS. No o-stage,
   no phase-boundary stall.
 - output is stored bf16 (host upcasts); halves the out DMA.

Precision: bf16 operands everywhere (fp8 E4M3 measured 3.3-4e-2 absmax rel
err for even one GEMM -- over the 2e-2 budget), fp32 accumulate.

Per-core layouts (i = input feature, o = q/k/v feature, j = out feature):
    ht  [i, m] = h[b].T          wq/wk/wv [i, o] = W.T        wo [o, j] = Wo.T
    k = ht.T @ wk : [m, o]       v : [m, o]      (partition = m)
    St_h = v_h^T @ k_h : [64,64] packed in two [128,512] PSUM banks
    woS = blkdiag(St_h) @ wo : [d, j]   qT = wq.T @ ht : [d, m]
    out = qT.T @ woS : [m, j]    (partition = m, DMA'd straight out)
"""

import numpy as np
import ml_dtypes

import concourse.bass as bass
import concourse.mybir as mybir
import concourse.tile as tile
from concourse import bacc
from concourse.bass_utils import run_bass_kernel_spmd

F32 = mybir.dt.float32
BF16 = mybir.dt.bfloat16

P = 128          # partitions
H = 1024         # model dim
M = 1024         # sequence length
NT = H // P      # 8 tiles of 128 along any 1024 dim
D = 64           # head dim
NH = 16          # heads
NC = 8           # cores
FD = 512         # matmul moving free dim
WARMUP_MM = 6    # PE warmup matmuls (bridge until the first DMA pair lands)

_CACHE = {}


def _build():
    nc = bacc.Bacc("TRN2", target_bir_lowering=False, debug=False,
                   enable_asserts=False, enable_partition_id=False)

    ht_d = nc.dram_tensor("ht", [H, M], BF16, kind="ExternalInput")
    wq_d = nc.dram_tensor("wq", [H, H], BF16, kind="ExternalInput")
    wk_d = nc.dram_tensor("wk", [H, H], BF16, kind="ExternalInput")
    wv_d = nc.dram_tensor("wv", [H, H], BF16, kind="ExternalInput")
    wo_d = nc.dram_tensor("wo", [H, H], BF16, kind="ExternalInput")
    out_d = nc.dram_tensor("out", [M, H], BF16, kind="ExternalOutput")

    with tile.TileContext(nc) as tc:
        with tc.tile_pool(name="sb", bufs=1) as sb, \
             tc.tile_pool(name="ps", bufs=1, space="PSUM") as ps:

            # ---- PE warmup on bank 7: dep-free matmuls issued while the
            # first DMAs stream in. Operands are framework const tensors ----
            wu_lhs = nc.const_aps.tensor(1.0, [P, P], BF16)
            wu_rhs = nc.const_aps.tensor(1.0, [P, FD], BF16)
            wu_ps = ps.tile([P, FD], F32, tag="b7", name="wu_ps")
            for _ in range(WARMUP_MM):
                nc.tensor.matmul(wu_ps[:], wu_lhs, wu_rhs,
                                 start=True, stop=True)

            # ---- loads: ramp-critical chunks interleaved across BOTH HW
            # DMA queues in the need-order of the ci-outer chains (pair ci =
            # ht_ci + wk_ci half0); the scalar HWDGE queue spins up ~1us
            # later than sync, so it gets the later-needed half of each
            # pair. wk half1 (co=1 chains), wv, wq, wo stream in after ----
            ht_s = [sb.tile([P, M], BF16, tag=f"ht{i}", name=f"ht{i}")
                    for i in range(NT)]
            wk_s = [sb.tile([P, H], BF16, tag=f"wa{i}", name=f"wk{i}")
                    for i in range(NT)]

            def ld_ht(q, i, lo=0, hi=M):
                q.dma_start(ht_s[i][:, lo:hi],
                            ht_d.ap()[P * i:P * i + P, lo:hi])

            def ld_w(q, dst, src, i, lo=0, hi=H):
                q.dma_start(dst[i][:, lo:hi], src.ap()[P * i:P * i + P, lo:hi])

            # tiny head transfer on scalar: absorbs the queue's first-packet
            # spin-up latency so the real ramp chunks stream right behind
            ld_ht(nc.sync, 0, 0, FD)
            ld_ht(nc.scalar, 0, FD, FD + 16)
            ld_ht(nc.scalar, 0, FD + 16, M)
            ld_w(nc.sync, wk_s, wk_d, 0, 0, FD)
            ld_w(nc.scalar, wk_s, wk_d, 1, 0, FD)
            for i in range(1, NT):
                ld_ht(nc.sync if i % 2 else nc.scalar, i)
                if i + 1 < NT:
                    # opposite parity from ht_{i+1}: each (ht, wk-h0) pair
                    # is split across the two queues, so pairs complete in
                    # lockstep instead of alternating whole-pair per queue
                    ld_w(nc.sync if i % 2 else nc.scalar,
                         wk_s, wk_d, i + 1, 0, FD)
            for i in range(NT):
                nc.scalar.dma_start(wk_s[i][:, FD:H],
                                    wk_d.ap()[P * i:P * i + P, FD:H])
            wv_s, wq_s = [], []
            for i in range(NT):
                v_t = sb.tile([P, H], BF16, tag=f"wb{i}", name=f"wv{i}")
                nc.scalar.dma_start(v_t[:], wv_d.ap()[P * i:P * i + P, :])
                wv_s.append(v_t)
                q_t = sb.tile([P, H], BF16, tag=f"wq{i}", name=f"wq{i}")
                nc.sync.dma_start(q_t[:], wq_d.ap()[P * i:P * i + P, :])
                wq_s.append(q_t)

            # ---- phase 1a: k projection ----
            # ci-outer ramp: 7 chains (tm=0..6, co=0) open on banks 0-6; MM
            # (tm, ci) needs only the DMA chunks of pair ci; chains 0-3 read
            # the ht_ci first half, 4-6 the second.
            k_tiles = [sb.tile([P, H], BF16, tag=f"k{t}", name=f"k{t}")
                       for t in range(NT)]
            acc = [ps.tile([P, FD], F32, tag=f"b{t}", name=f"ka{t}")
                   for t in range(7)]
            for ci in range(NT):
                for t in range(7):
                    nc.tensor.matmul(
                        acc[t][:],
                        ht_s[ci][:, P * t:P * t + P],
                        wk_s[ci][:, 0:FD],
                        start=(ci == 0), stop=(ci == NT - 1),
                        skip_group_check=True,
                    )
                # dep-free filler between early groups: absorbs DMA-arrival
                # jitter and keeps the HAM activity window alive if a chunk
                # is late (an idle gap here delays the 2.4GHz unthrottle)
                if ci < 2:
                    for _ in range(2 - ci):
                        nc.tensor.matmul(wu_ps[:], wu_lhs, wu_rhs,
                                         start=True, stop=True)
            for t in range(7):
                nc.vector.tensor_copy(k_tiles[t][:, 0:FD], acc[t][:])

            # remaining k chains run sequentially (all data resident by now);
            # each bank's evacuation cast hides under the next chain
            def kchain(tm, co, bank):
                p_t = ps.tile([P, FD], F32, tag=bank, name=f"kb{tm}_{co}")
                for ci in range(NT):
                    nc.tensor.matmul(
                        p_t[:],
                        ht_s[ci][:, P * tm:P * tm + P],
                        wk_s[ci][:, FD * co:FD * co + FD],
                        start=(ci == 0), stop=(ci == NT - 1),
                    )
                nc.vector.tensor_copy(
                    k_tiles[tm][:, FD * co:FD * co + FD], p_t[:])

            kchain(7, 0, "b7")
            for tm in range(NT):
                kchain(tm, 1, f"b{tm}")

            # ---- phase 1b: v projection + S^T accumulation ----
            # One matmul per (m-tile, head pair g): v_pair^T @ k_pair gives a
            # 2x2 head block; diagonal blocks are S_2g^T / S_2g+1^T. Pairs
            # 0-3 accumulate in s_psA, 4-7 in s_psB. Interleaved per-pair
            # groups share the banks; zero once, accumulate with start=False.
            s_psA = ps.tile([P, FD], F32, tag="b0", name="s_psA")
            s_psB = ps.tile([P, FD], F32, tag="b1", name="s_psB")
            nc.vector.memset(s_psA[:], 0.0)
            nc.vector.memset(s_psB[:], 0.0)
            vb = 0
            for tm in range(NT):
                k_t = k_tiles[tm]
                v_t = sb.tile([P, H], BF16, tag="vv", bufs=4, name=f"v{tm}")
                for co in range(2):
                    p_t = ps.tile([P, FD], F32, tag=f"b{2 + vb % 6}",
                                  name=f"pv{tm}_{co}")
                    vb += 1
                    for ci in range(NT):
                        nc.tensor.matmul(
                            p_t[:],
                            ht_s[ci][:, P * tm:P * tm + P],
                            wv_s[ci][:, FD * co:FD * co + FD],
                            start=(ci == 0), stop=(ci == NT - 1),
                        )
                    nc.vector.tensor_copy(v_t[:, FD * co:FD * co + FD], p_t[:])
                    for g in range(4 * co, 4 * co + 4):
                        bank = s_psA if g < 4 else s_psB
                        cc = P * (g % 4)
                        nc.tensor.matmul(
                            bank[:, cc:cc + P],
                            v_t[:, P * g:P * g + P],
                            k_t[:, P * g:P * g + P],
                            start=False, stop=(tm == NT - 1),
                            skip_group_check=True,
                        )

            # ---- wo loads (reuse wa slots once wk is done) ----
            wo_s = []
            for i in range(NT):
                o_t = sb.tile([P, H], BF16, tag=f"wa{i}", name=f"wo{i}")
                (nc.sync if i % 2 else nc.scalar).dma_start(
                    o_t[:], wo_d.ap()[P * i:P * i + P, :])
                wo_s.append(o_t)

            # ---- block-diagonal S^T tiles: bd[g] = diag(S_2g^T, S_2g+1^T)
            # zero-padded, the lhsT of the wo-fold matmuls ----
            bd = []
            for g in range(NT):
                b_t = sb.tile([P, P], BF16, tag=f"bd{g}", name=f"bd{g}")
                nc.vector.memset(b_t[:], 0.0)
                s_bank = s_psA if g < 4 else s_psB
                cc = P * (g % 4)
                nc.vector.tensor_copy(b_t[0:D, 0:D], s_bank[0:D, cc:cc + D])
                nc.vector.tensor_copy(b_t[D:P, D:P],
                                      s_bank[D:P, cc + D:cc + P])
                bd.append(b_t)

            # ---- phase 2: qT projection; the wo-fold matmuls
            # (woS = blkdiag(S^T) @ wo) slot in after the first q chains so
            # the vector engine has time to build the bd tiles ----
            qt_tiles = [None] * NT
            pb = [0]

            def nextbank():
                t = f"b{2 + pb[0] % 6}"
                pb[0] += 1
                return t

            def emit_q(to):
                q_t = sb.tile([P, M], BF16, tag=f"qt{to}", name=f"qt{to}")
                for cm in range(2):
                    p_t = ps.tile([P, FD], F32, tag=nextbank(),
                                  name=f"pq{to}_{cm}")
                    for ci in range(NT):
                        nc.tensor.matmul(
                            p_t[:],
                            wq_s[ci][:, P * to:P * to + P],
                            ht_s[ci][:, FD * cm:FD * cm + FD],
                            start=(ci == 0), stop=(ci == NT - 1),
                        )
                    nc.vector.tensor_copy(q_t[:, FD * cm:FD * cm + FD], p_t[:])
                qt_tiles[to] = q_t

            ws_tiles = []

            def emit_ws(to):
                w_t = sb.tile([P, H], BF16, tag=f"ws{to}", name=f"ws{to}")
                for cj in range(2):
                    p_t = ps.tile([P, FD], F32, tag=nextbank(),
                                  name=f"pw{to}_{cj}")
                    nc.tensor.matmul(
                        p_t[:],
                        bd[to][:],
                        wo_s[to][:, FD * cj:FD * cj + FD],
                        start=True, stop=True,
                    )
                    nc.vector.tensor_copy(w_t[:, FD * cj:FD * cj + FD], p_t[:])
                ws_tiles.append(w_t)

            # interleave: each ws pair (0.4us of PE, 1.4us of casts) hides
            # under a q chain (3.5us of PE) so bank evacuations keep up
            emit_q(0)
            emit_q(1)
            emit_ws(0), emit_ws(1)
            emit_q(2)
            emit_ws(2), emit_ws(3)
            emit_q(3)
            emit_ws(4), emit_ws(5)
            emit_q(4)
            emit_ws(6), emit_ws(7)
            for to in range(5, NT):
                emit_q(to)

            # ---- phase 3: out = qT.T @ woS (bf16 out, host upcasts) ----
            # the very last 512-col chunk runs as two [128,256] chains so the
            # first half's cast+store hides under the second half's matmuls
            # and the tail gates on a [128,256] cast + 64KB transfer
            ob = [0]

            def out_chain(tm, lo, hi, q, cast_eng=None):
                o_sb = out_sb[tm]
                p_t = ps.tile([P, hi - lo], F32, tag=f"b{ob[0] % 8}",
                              name=f"pf{tm}_{lo}")
                ob[0] += 1
                for to in range(NT):
                    nc.tensor.matmul(
                        p_t[:],
                        qt_tiles[to][:, P * tm:P * tm + P],
                        ws_tiles[to][:, lo:hi],
                        start=(to == 0), stop=(to == NT - 1),
                    )
                if cast_eng is None:
                    nc.vector.tensor_copy(o_sb[:, lo:hi], p_t[:])
                else:
                    cast_eng.copy(o_sb[:, lo:hi], p_t[:])
                q.dma_start(out_d.ap()[P * tm:P * tm + P, lo:hi],
                            o_sb[:, lo:hi])

            out_sb = [sb.tile([P, H], BF16, tag=f"wb{t}", name=f"osb{t}")
                      for t in range(NT)]
            for tm in range(NT):
                out_chain(tm, 0, FD, nc.scalar)
                if tm == NT - 1:
                    out_chain(tm, FD, FD + 256, nc.sync)
                    out_chain(tm, FD + 256, FD + 384, nc.scalar,
                              cast_eng=nc.scalar)
                    out_chain(tm, FD + 384, H, nc.sync)
                else:
                    out_chain(tm, FD, H, nc.sync)

    nc.compile()
    return nc


def _get_nc():
    if "nc" not in _CACHE:
        _CACHE["nc"] = _build()
    return _CACHE["nc"]


def _run(h, Wq, Wk, Wv, Wo, trace=False):
    nc = _get_nc()
    bf16 = ml_dtypes.bfloat16
    wq = np.ascontiguousarray(np.asarray(Wq).T).astype(bf16)
    wk = np.ascontiguousarray(np.asarray(Wk).T).astype(bf16)
    wv = np.ascontiguousarray(np.asarray(Wv).T).astype(bf16)
    wo = np.ascontiguousarray(np.asarray(Wo).T).astype(bf16)
    in_maps = []
    for b in range(NC):
        in_maps.append({
            "ht": np.ascontiguousarray(np.asarray(h[b]).T).astype(bf16),
            "wq": wq, "wk": wk, "wv": wv, "wo": wo,
        })
    res = run_bass_kernel_spmd(nc, in_maps, core_ids=list(range(NC)), trace=trace)
    out = np.stack(
        [np.asarray(res.results[b]["out"]).astype(np.float32)
         for b in range(NC)], axis=0)
    return out, res


def kernel(h, key_pe, Wq, Wk, Wv, Wo):
    # key_pe only feeds the reference's dead softmax branch; unused.
    out, _ = _run(h, Wq, Wk, Wv, Wo)
    return out
